# revision 1
# baseline (speedup 1.0000x reference)
"""Dilated self-attention Trainium2 kernel (8-core SPMD).

Problem (hardcoded): x [4, 8192, 256], Wq/Wk/Wv [256, 256] f32.
WS=[2048,4096,8192], RS=[1,2,4], HEAD_IDX=0 -> every config has segment
length 2048 after dilation; 28 segments total.

Sharding: core = (b, h) with b in 0..3, h in 0..1. Core (b,h) owns output
tokens [4096h, 4096h+4096) of batch b and computes the 4 attention
segments that contribute to them:
  seg0 = config1 seg 2h   (tokens 4096h+[0,2048))
  seg1 = config1 seg 2h+1 (tokens 4096h+[2048,4096))
  seg2 = config2 seg h    (tokens 4096h+(0,2,4,...) -- 2048 even rows)
  seg3 = config3 seg 0    (tokens 0::4 over the whole batch, computed
                           fully on both cores of the pair; each core
                           uses only its half of the rows, selected with
                           a runtime register offset so the SPMD program
                           is identical across cores)
Per-token combine (sum of unnormalized outputs / sum of denominators)
is then fully core-local; no collectives.

Layout tricks: host passes x pre-transposed per segment (xsT [4,256,2048])
and transposed weights WqT/WkT; the kernel computes
  GT = Wk @ Wq^T           (once)
  HT(seg) = GT^T??  -- H^T = G @ X^T  via lhsT=GT slices
  scores_T[k,q] = H^T(:,k)^T. X^T = (X G^T X^T)^T block
so no on-device transposes are needed anywhere. The output is produced
transposed ([256, 4096]) and un-transposed on the host.
"""

import os
import numpy as np

import concourse.bass as bass
import concourse.mybir as mybir
import concourse.tile as tile
from concourse import bacc
from concourse.bass_utils import run_bass_kernel_spmd
from concourse.masks import make_identity

F32 = mybir.dt.float32
F32R = mybir.dt.float32r
I32 = mybir.dt.int32
AF = mybir.ActivationFunctionType

B, N, C, D = 4, 8192, 256, 256
SEG = 2048          # segment length (rows) for every config
P = 128             # partitions
NT = SEG // P       # 16 k-tiles per segment
QST = 512           # q supertile width
NJ = SEG // QST     # 4 q supertiles per segment
HALF = N // 2       # 4096 tokens owned per core
NSEG = 4            # segments per core
MASK_VAL = -20000.0
SCALE = 1.0 / 16.0  # 1/sqrt(D)

ABL = os.environ.get("ABL", "")
USE_REPS_LOOP = True
USE_F32R = True     # fp32r matmuls (4x faster PE, slightly reduced precision)
MMDT = F32R if USE_F32R else F32
BF16 = mybir.dt.bfloat16
USE_BF16_EV = False  # bf16 matmuls measured slower than f32r on this HW
EDT = BF16 if USE_BF16_EV else MMDT


def _mm_dt(ap):
    return ap


def _emit(tc):
    nc = tc.nc

    xsT_d = nc.dram_tensor("xsT", [NSEG, C, SEG], MMDT, kind="ExternalInput").ap()
    wqT_d = nc.dram_tensor("wqT", [D, C], MMDT, kind="ExternalInput").ap()
    wkT_d = nc.dram_tensor("wkT", [D, C], MMDT, kind="ExternalInput").ap()
    wv_d = nc.dram_tensor("wv", [C, D], MMDT, kind="ExternalInput").ap()
    c3off_d = nc.dram_tensor("c3off", [1, 1], I32, kind="ExternalInput").ap()
    reps_d = nc.dram_tensor("reps", [1, 1], I32, kind="ExternalInput").ap()
    outT_d = nc.dram_tensor("outT", [C, HALF], F32, kind="ExternalOutput").ap()

    import contextlib
    ctx = contextlib.ExitStack()
    with ctx:
        consts = ctx.enter_context(tc.tile_pool(name="consts", bufs=1))
        big = ctx.enter_context(tc.tile_pool(name="big", bufs=1))
        xt_pool = ctx.enter_context(tc.tile_pool(name="xt", bufs=2))
        e_pool = ctx.enter_context(tc.tile_pool(name="e", bufs=4))
        pr_pool = ctx.enter_context(tc.tile_pool(name="pr", bufs=2))
        ps_sc = ctx.enter_context(tc.tile_pool(name="ps_sc", bufs=2, space="PSUM"))
        ps_o = ctx.enter_context(tc.tile_pool(name="ps_o", bufs=3, space="PSUM"))
        ps_den = ctx.enter_context(tc.tile_pool(name="ps_den", bufs=1, space="PSUM"))

        # ---- constants ----
        wqT_sb = [consts.tile([P, C], MMDT, tag=f"wqT{i}", name=f"wqT{i}") for i in range(2)]
        wkT_sb = [consts.tile([P, C], MMDT, tag=f"wkT{i}", name=f"wkT{i}") for i in range(2)]
        wv_sb = [consts.tile([P, D], MMDT, tag=f"wv{i}", name=f"wv{i}") for i in range(2)]
        for i in range(2):
            nc.sync.dma_start(wqT_sb[i], wqT_d[P * i:P * (i + 1), :])
            nc.sync.dma_start(wkT_sb[i], wkT_d[P * i:P * (i + 1), :])
            nc.sync.dma_start(wv_sb[i], wv_d[P * i:P * (i + 1), :])

        c3off_sb = consts.tile([1, 1], I32, tag="c3off")
        nc.sync.dma_start(c3off_sb, c3off_d)
        reps_sb = consts.tile([1, 1], I32, tag="reps")
        nc.sync.dma_start(reps_sb, reps_d)

        ident_f = consts.tile([P, QST], F32, tag="mscratch", name="ident_f")
        make_identity(nc, ident_f[:, 0:P])
        ident = consts.tile([P, P], EDT, tag="ident")
        nc.vector.tensor_copy(ident, ident_f[:, 0:P])

        ones_f = consts.tile([P, P], F32, tag="ones_f")
        nc.vector.memset(ones_f, 1.0)
        ones_col = consts.tile([P, P], EDT, tag="ones_col")
        nc.vector.tensor_copy(ones_col, ones_f)

        # diagonal-region additive masks M_t [128, 512]:
        # M_t[kr, qc] = 0 if qc >= kr + 128*t else MASK_VAL
        masks = []
        for t in range(NJ):
            mf = consts.tile([P, QST], F32, tag="mscratch", name=f"mask_f{t}")
            nc.gpsimd.memset(mf, 0.0)
            nc.gpsimd.affine_select(
                out=mf, in_=mf, compare_op=mybir.AluOpType.is_ge,
                fill=MASK_VAL, base=-P * t, channel_multiplier=-1,
                pattern=[[1, QST]],
            )
            m = consts.tile([P, QST], EDT, tag=f"mask{t}", name=f"mask{t}")
            nc.vector.tensor_copy(m, mf)
            masks.append(m)

        # GT = Wk @ Wq^T  [256, 256]  (= (Wq Wk^T)^T)
        GT_sb = [consts.tile([P, C], MMDT, tag=f"GT{i}", name=f"GT{i}") for i in range(2)]
        for a in range(2):  # output row chunk
            ps = ps_sc.tile([P, 2, QST], F32, tag="psc", name="gtps")[:, 0, 0:C]
            for dch in range(2):
                nc.tensor.matmul(
                    ps, _mm_dt(wkT_sb[dch][:, P * a:P * (a + 1)]), _mm_dt(wqT_sb[dch]),
                    start=(dch == 0), stop=(dch == 1))
            nc.vector.tensor_copy(GT_sb[a], ps)

        # ---- persistent per-iteration state ----
        # oT[s][c]: unnormalized attention output, transposed: [128, 2048] per
        # (segment s, feature chunk c). den[s]: [1, 2048].
        oT = big.tile([P, NSEG, 2, SEG], F32, tag="oT")
        # all denominators in one partition-0 row: seg s at offset s*SEG
        den = big.tile([1, NSEG, SEG], F32, tag="den")

        c3v = nc.values_load(c3off_sb, min_val=0, max_val=SEG // 2, skip_runtime_bounds_check=True)
        reps_v = nc.values_load(reps_sb, min_val=1, max_val=10000, skip_runtime_bounds_check=True)

        def body(_iv):
            for s in (2, 3, 0, 1):
                # load x^T for this segment
                xT = [xt_pool.tile([P, SEG], MMDT, tag=f"xT{c}", name=f"xT{c}") for c in range(2)]
                for c in range(2):
                    nc.sync.dma_start(xT[c], xsT_d[s, P * c:P * (c + 1), :])

                # HT = G @ X^T : [256, 2048]; lhsT = GT slices
                HT = [xt_pool.tile([P, SEG], MMDT, tag=f"HT{c}", name=f"HT{c}", bufs=1) for c in range(2)]
                for fo in range(2):
                    for rp in range(NJ // 2):
                        ps = ps_sc.tile([P, 2, QST], F32, tag="psc")
                        for idx in range(2):
                            r = 2 * rp + idx
                            for fi in range(2):
                                nc.tensor.matmul(
                                    ps[:, idx, :],
                                    _mm_dt(GT_sb[fi][:, P * fo:P * (fo + 1)]),
                                    _mm_dt(xT[fi][:, QST * r:QST * (r + 1)]),
                                    start=(fi == 0), stop=(fi == 1))
                        nc.vector.tensor_copy(
                            HT[fo][:, QST * 2 * rp:QST * 2 * (rp + 1)],
                            ps.rearrange("p a b -> p (a b)"))

                # V = X @ Wv : [2048, 256] natural (16 tiles of [128, 256])
                V = xt_pool.tile([P, NT, D], EDT, tag="V", bufs=1)
                for kq in range(NT // 4):
                    ps = ps_sc.tile([P, 2, QST], F32, tag="psc")
                    psf = ps.rearrange("p a b -> p (a b)")
                    for idx in range(4):
                        kt = 4 * kq + idx
                        for fi in range(2):
                            nc.tensor.matmul(
                                psf[:, D * idx:D * (idx + 1)],
                                _mm_dt(xT[fi][:, P * kt:P * (kt + 1)]),
                                _mm_dt(wv_sb[fi]),
                                start=(fi == 0), stop=(fi == 1))
                    nc.vector.tensor_copy(
                        V[:, 4 * kq:4 * (kq + 1), :].rearrange("p a b -> p (a b)"),
                        psf)

                # attention: scores_T[k, q] blocks, flash accumulation over kt
                for j in range(NJ):
                    po = [ps_o.tile([P, QST], F32, tag="po", name=f"po{_i}") for _i in range(2)]
                    pd = ps_den.tile([P, QST], F32, tag="pd")
                    nkt = 4 * j + 4
                    for g in range(nkt // 2):
                        psc = ps_sc.tile([P, 2, QST], F32, tag="psc")
                        for idx in range(2):
                            kt = 2 * g + idx
                            diag = kt >= 4 * j
                            nc.tensor.matmul(
                                psc[:, idx, :],
                                _mm_dt(HT[0][:, P * kt:P * (kt + 1)]),
                                _mm_dt(xT[0][:, QST * j:QST * (j + 1)]),
                                start=True, stop=False)
                            nc.tensor.matmul(
                                psc[:, idx, :],
                                _mm_dt(HT[1][:, P * kt:P * (kt + 1)]),
                                _mm_dt(xT[1][:, QST * j:QST * (j + 1)]),
                                start=False, stop=not diag)
                            if diag:
                                nc.tensor.matmul(
                                    psc[:, idx, :], _mm_dt(ident),
                                    _mm_dt(masks[kt - 4 * j]),
                                    start=False, stop=True)
                        e = e_pool.tile([P, 2, QST], EDT, tag="e")
                        nc.scalar.activation(e, psc, AF.Exp, scale=SCALE)
                        for idx in range(2):
                            kt = 2 * g + idx
                            nc.tensor.matmul(
                                po[0], _mm_dt(V[:, kt, 0:P]), _mm_dt(e[:, idx, :]),
                                start=(kt == 0), stop=(kt == nkt - 1))
                            nc.tensor.matmul(
                                po[1], _mm_dt(V[:, kt, P:D]), _mm_dt(e[:, idx, :]),
                                start=(kt == 0), stop=(kt == nkt - 1))
                            nc.tensor.matmul(
                                pd, _mm_dt(ones_col), _mm_dt(e[:, idx, :]),
                                start=(kt == 0), stop=(kt == nkt - 1))
                    for c in range(2):
                        nc.vector.tensor_copy(oT[:, s, c, QST * j:QST * (j + 1)], po[c])
                    nc.vector.tensor_copy(den[:, s, QST * j:QST * (j + 1)], pd[0:1, :])

            # ---- combine ----
                if s in (0, 1):
                    _combine_seg(s)

        def _combine_seg(s):
            # fold config2 (even tokens) and config3 (every 4th) into seg s,
            # then divide by the summed denominator and store. Emitted right
            # after segment s so it overlaps the next segment's attention.
            for ch in range(NJ):
                lo = QST * ch
                g = SEG * s + lo            # token offset inside the half
                for c in range(2):
                    dst = oT[:, s, c, lo:lo + QST]
                    d2 = dst.rearrange("p (q two) -> p q two", two=2)[:, :, 0:1]
                    nc.vector.tensor_add(
                        d2, d2,
                        oT[:, 2, c, g // 2:g // 2 + QST // 2].rearrange(
                            "p (q one) -> p q one", one=1))
                    d4 = dst.rearrange("p (q four) -> p q four", four=4)[:, :, 0:1]
                    nc.vector.tensor_add(
                        d4, d4,
                        oT[:, 3, c, bass.ds(c3v + g // 4, QST // 4)].rearrange(
                            "p (q one) -> p q one", one=1))
                dstd = den[:, s, lo:lo + QST]
                dd2 = dstd.rearrange("p (q two) -> p q two", two=2)[:, :, 0:1]
                nc.vector.tensor_add(
                    dd2, dd2,
                    den[:, 2, g // 2:g // 2 + QST // 2].rearrange(
                        "p (q one) -> p q one", one=1))
                dd4 = dstd.rearrange("p (q four) -> p q four", four=4)[:, :, 0:1]
                nc.vector.tensor_add(
                    dd4, dd4,
                    den[:, 3, bass.ds(c3v + g // 4, QST // 4)].rearrange(
                        "p (q one) -> p q one", one=1))
                pr = pr_pool.tile([P, QST], F32, tag="pr")
                nc.gpsimd.partition_broadcast(pr, dstd)
                nc.vector.reciprocal(pr, pr)
                for c in range(2):
                    nc.vector.tensor_mul(
                        oT[:, s, c, lo:lo + QST], oT[:, s, c, lo:lo + QST], pr)
                    nc.sync.dma_start(
                        outT_d[P * c:P * (c + 1), g:g + QST],
                        oT[:, s, c, lo:lo + QST])

        if USE_REPS_LOOP:
            with tc.For_i(0, reps_v) as iv:
                body(iv)
        else:
            body(0)


_NC_CACHE = None


def _get_nc():
    global _NC_CACHE
    if _NC_CACHE is None:
        nc = bacc.Bacc("TRN2", target_bir_lowering=False, debug=False,
                       num_devices=8)
        with tile.TileContext(nc) as tc:
            _emit(tc)
        nc.compile()
        _NC_CACHE = nc
    return _NC_CACHE


def _make_in_maps(x, Wq, Wk, Wv, reps=1):
    wqT = np.ascontiguousarray(Wq.T)
    wkT = np.ascontiguousarray(Wk.T)
    wv = np.ascontiguousarray(Wv)
    in_maps = []
    for core in range(8):
        b, h = core // 2, core % 2
        xb = x[b]                                  # [8192, 256]
        xa = xb[HALF * h:HALF * (h + 1)]           # [4096, 256]
        segs = [
            xa[0:SEG],                             # config1 seg 2h
            xa[SEG:2 * SEG],                       # config1 seg 2h+1
            xa[0::2],                              # config2 seg h
            xb[0::4],                              # config3 (full)
        ]
        xsT = np.ascontiguousarray(
            np.stack([s.T for s in segs], axis=0), dtype=np.float32)
        in_maps.append({
            "xsT": xsT,
            "wqT": wqT,
            "wkT": wkT,
            "wv": wv,
            "c3off": np.array([[(SEG // 2) * h]], dtype=np.int32),
            "reps": np.array([[reps]], dtype=np.int32),
        })
    return in_maps


def run_cores(x, Wq, Wk, Wv, reps=1):
    nc = _get_nc()
    in_maps = _make_in_maps(x, Wq, Wk, Wv, reps=reps)
    res = run_bass_kernel_spmd(nc, in_maps, core_ids=list(range(8)))
    return res


def kernel(x, Wq, Wk, Wv):
    x = np.asarray(x, dtype=np.float32)
    res = run_cores(x, np.asarray(Wq, np.float32), np.asarray(Wk, np.float32),
                    np.asarray(Wv, np.float32))
    out = np.empty((B, N, D), dtype=np.float32)
    for core in range(8):
        b, h = core // 2, core % 2
        out[b, HALF * h:HALF * (h + 1), :] = res.results[core]["outT"].T
    return out



# revision 3
# speedup vs baseline: 1.1937x; 1.1937x over previous
"""Dilated self-attention Trainium2 kernel (8-core SPMD), v2.

Problem (hardcoded): x [4, 8192, 256], Wq/Wk/Wv [256, 256] f32.
WS=[2048,4096,8192], RS=[1,2,4], HEAD_IDX=0 -> every config has segment
length 2048 after dilation; 28 segments total.

Sharding: core = (b, h) with b in 0..3, h in 0..1. Core (b,h) owns output
tokens [4096h, 4096h+4096) of batch b and computes the 4 attention
segments that contribute to them (seg3 = config3 computed fully on both
cores of a pair; each uses its half of the rows via a runtime offset).

Optimizations vs the f32r v1 (305us -> ~234us profiled single-shot;
PE-bound at ~80% TensorMatrix occupancy):
  - all matmul operands in bf16 (halves LDWEIGHTS time + input DMA);
    psum stays f32.  exp/softmax weights tolerate bf16 easily at the
    2e-2 rel-err gate (measured rel_fro ~4.6e-3).
  - hybrid causal blocking: off-diagonal k-tiles at 512-wide q blocks;
    the 4-k-tile diagonal region is split into two 256-wide q halves so
    ~25% of diagonal-block matmul rows are skipped instead of masked.
  - causal masking via gpsimd.affine_select zeroing e in SBUF (frees
    the PE mask matmuls entirely; no mask constants, no identity).
  - denominator: e pairs are pre-summed on DVE (bf16, cheap) so the
    ones-matmul runs once per pair instead of once per k-tile.
  - software pipelining: PV matmuls of pair g are emitted after the
    score matmuls of pair g+1 so the PE never waits on the Exp.
  - combine fused per 512-token chunk right after that chunk's PV
    finishes; reciprocal via reciprocal_approx_fast on the [1,512] den
    row, broadcast across partitions with a PE ones-matmul (gpsimd
    partition_broadcast head-of-line-blocks the affine_selects); the
    broadcast+divide+store half is deferred one j-iteration so the PE
    queue always has score matmuls ahead of it.
  - batched input DMAs (each dma_start costs ~0.6us of issue time on
    the sync engine): one packed weights tensor, xT prefetch bufs=4.
  - outputs written in bf16 (halved output DMA), upcast on host.

Timing note: device clock state varies ~20% between sessions; only
back-to-back in-session comparisons are meaningful.
"""

import numpy as np
import ml_dtypes

import concourse.bass as bass
import concourse.mybir as mybir
import concourse.tile as tile
from concourse import bacc
from concourse.bass_utils import run_bass_kernel_spmd

F32 = mybir.dt.float32
BF16 = mybir.dt.bfloat16
I32 = mybir.dt.int32
AF = mybir.ActivationFunctionType
GE = mybir.AluOpType.is_ge

B, N, C, D = 4, 8192, 256, 256
SEG = 2048          # segment length (rows) for every config
P = 128             # partitions
NT = SEG // P       # 16 k-tiles per segment
QST = 512           # q supertile width
HQ = QST // 2       # 256: diagonal-region q half width
NJ = SEG // QST     # 4 q supertiles per segment
HALF = N // 2       # 4096 tokens owned per core
NSEG = 4            # segments per core
SCALE = 1.0 / 16.0  # 1/sqrt(D)


def _emit(tc):
    nc = tc.nc

    xsT_d = nc.dram_tensor("xsT", [NSEG, C, SEG], BF16, kind="ExternalInput").ap()
    # wall: [128, 6, 256] partition-major pack of wqT(2) | wkT(2) | wv(2)
    wall_d = nc.dram_tensor("wall", [P, 6, C], BF16, kind="ExternalInput").ap()
    meta_d = nc.dram_tensor("meta", [1, 1], I32, kind="ExternalInput").ap()
    outT_d = nc.dram_tensor("outT", [C, HALF], BF16, kind="ExternalOutput").ap()

    import contextlib
    ctx = contextlib.ExitStack()
    with ctx:
        consts = ctx.enter_context(tc.tile_pool(name="consts", bufs=1))
        big = ctx.enter_context(tc.tile_pool(name="big", bufs=1))
        xt_pool = ctx.enter_context(tc.tile_pool(name="xt", bufs=2))
        e_pool = ctx.enter_context(tc.tile_pool(name="e", bufs=4))
        e01_pool = ctx.enter_context(tc.tile_pool(name="e01", bufs=4))
        pr_pool = ctx.enter_context(tc.tile_pool(name="pr", bufs=2))
        ob_pool = ctx.enter_context(tc.tile_pool(name="ob", bufs=3))
        ps_sc = ctx.enter_context(tc.tile_pool(name="ps_sc", bufs=2, space="PSUM"))
        ps_o = ctx.enter_context(tc.tile_pool(name="ps_o", bufs=1, space="PSUM"))
        ps_den = ctx.enter_context(tc.tile_pool(name="ps_den", bufs=1, space="PSUM"))
        ps_pr = ctx.enter_context(tc.tile_pool(name="ps_pr", bufs=1, space="PSUM"))

        # ---- constants (batched DMAs: issue cost on sync is ~0.6us each) ----
        meta_sb = consts.tile([1, 1], I32, tag="meta")
        nc.sync.dma_start(meta_sb, meta_d)
        wall = consts.tile([P, 6, C], BF16, tag="wall")
        nc.sync.dma_start(wall, wall_d)
        wqT_sb = [wall[:, i, :] for i in range(2)]
        wkT_sb = [wall[:, 2 + i, :] for i in range(2)]
        wv_sb = [wall[:, 4 + i, :] for i in range(2)]
        c3off_sb = meta_sb[:, 0:1]

        ones_col = consts.tile([P, P], BF16, tag="ones_col")
        nc.vector.memset(ones_col, 1.0)
        ones_row_f = consts.tile([1, P], F32, tag="ones_row_f")
        nc.vector.memset(ones_row_f, 1.0)
        ones_row = consts.tile([1, P], mybir.dt.float32r, tag="ones_row")
        nc.vector.tensor_copy(ones_row, ones_row_f)

        # GT = Wk @ Wq^T  [256, 256]  (= (Wq Wk^T)^T)
        GT_sb = [consts.tile([P, C], BF16, tag=f"GT{i}", name=f"GT{i}") for i in range(2)]
        for a in range(2):  # output row chunk
            ps = ps_sc.tile([P, 2, QST], F32, tag="psc", name="gtps")[:, 0, 0:C]
            for dch in range(2):
                nc.tensor.matmul(
                    ps, wkT_sb[dch][:, P * a:P * (a + 1)], wqT_sb[dch],
                    start=(dch == 0), stop=(dch == 1))
            nc.vector.tensor_copy(GT_sb[a], ps)

        # ---- persistent per-iteration state ----
        # oT[s][c]: unnormalized attention output, transposed: [128, 2048] per
        # (segment s, feature chunk c). den[s]: [1, 2048].
        oT = big.tile([P, NSEG, 2, SEG], F32, tag="oT")
        den = big.tile([1, NSEG, SEG], F32, tag="den")

        c3v = nc.values_load(c3off_sb, min_val=0, max_val=SEG // 2, skip_runtime_bounds_check=True)

        pending_cb = []

        def _flush_cb():
            while pending_cb:
                pending_cb.pop(0)()

        def _combine_chunk(s, j):
            # fold config2 (even tokens) and config3 (every 4th) into chunk j
            # of seg s, divide by the summed denominator, store (bf16).
            # The division/DMA half is deferred one j-iteration so its PE
            # broadcast matmul queues behind fresh score matmuls instead of
            # head-of-line-blocking the PE on the DVE chain.
            lo = QST * j
            g = SEG * s + lo            # token offset inside the half
            dstd = den[:, s, lo:lo + QST]
            dd2 = dstd.rearrange("p (q two) -> p q two", two=2)[:, :, 0:1]
            nc.vector.tensor_add(
                dd2, dd2,
                den[:, 2, g // 2:g // 2 + QST // 2].rearrange(
                    "p (q one) -> p q one", one=1))
            dd4 = dstd.rearrange("p (q four) -> p q four", four=4)[:, :, 0:1]
            nc.vector.tensor_add(
                dd4, dd4,
                den[:, 3, bass.ds(c3v + g // 4, QST // 4)].rearrange(
                    "p (q one) -> p q one", one=1))
            pr1 = pr_pool.tile([1, QST], F32, tag="pr1", name="pr1")
            nc.vector.reciprocal_approx_fast(pr1, dstd)
            pr1r = pr_pool.tile([1, QST], mybir.dt.float32r, tag="pr1r",
                                name="pr1r")
            nc.vector.tensor_copy(pr1r, pr1)
            # broadcast 1/den across partitions on the PE (a gpsimd
            # partition_broadcast here head-of-line-blocks the affine_selects)
            for c in range(2):
                dst = oT[:, s, c, lo:lo + QST]
                d2 = dst.rearrange("p (q two) -> p q two", two=2)[:, :, 0:1]
                nc.vector.tensor_add(
                    d2, d2,
                    oT[:, 2, c, g // 2:g // 2 + QST // 2].rearrange(
                        "p (q one) -> p q one", one=1))
                d4 = dst.rearrange("p (q four) -> p q four", four=4)[:, :, 0:1]
                nc.vector.tensor_add(
                    d4, d4,
                    oT[:, 3, c, bass.ds(c3v + g // 4, QST // 4)].rearrange(
                        "p (q one) -> p q one", one=1))

            def part_b():
                pr_ps = ps_pr.tile([P, QST], F32, tag="prps")
                nc.tensor.matmul(pr_ps, ones_row, pr1r, start=True, stop=True)
                for c in range(2):
                    ob = ob_pool.tile([P, QST], BF16, tag="ob", name=f"ob{c}")
                    nc.vector.tensor_mul(ob, oT[:, s, c, lo:lo + QST], pr_ps)
                    nc.sync.dma_start(outT_d[P * c:P * (c + 1), g:g + QST], ob)
            pending_cb.append(part_b)

        def body(_iv):
            for s in (2, 3, 0, 1):
                # load x^T for this segment
                xT = [xt_pool.tile([P, SEG], BF16, tag=f"xT{c}", name=f"xT{c}", bufs=4) for c in range(2)]
                for c in range(2):
                    nc.sync.dma_start(xT[c], xsT_d[s, P * c:P * (c + 1), :])

                # HT = G @ X^T : [256, 2048]; lhsT = GT slices
                HT = [xt_pool.tile([P, SEG], BF16, tag=f"HT{c}", name=f"HT{c}", bufs=1) for c in range(2)]
                for fo in range(2):
                    for rp in range(NJ // 2):
                        ps = ps_sc.tile([P, 2, QST], F32, tag="psc")
                        for idx in range(2):
                            r = 2 * rp + idx
                            for fi in range(2):
                                nc.tensor.matmul(
                                    ps[:, idx, :],
                                    GT_sb[fi][:, P * fo:P * (fo + 1)],
                                    xT[fi][:, QST * r:QST * (r + 1)],
                                    start=(fi == 0), stop=(fi == 1))
                        nc.vector.tensor_copy(
                            HT[fo][:, QST * 2 * rp:QST * 2 * (rp + 1)],
                            ps.rearrange("p a b -> p (a b)"))

                # V = X @ Wv : [2048, 256] natural (16 tiles of [128, 256])
                V = xt_pool.tile([P, NT, D], BF16, tag="V", bufs=1)
                for kq in range(NT // 4):
                    ps = ps_sc.tile([P, 2, QST], F32, tag="psc")
                    psf = ps.rearrange("p a b -> p (a b)")
                    for idx in range(4):
                        kt = 4 * kq + idx
                        for fi in range(2):
                            nc.tensor.matmul(
                                psf[:, D * idx:D * (idx + 1)],
                                xT[fi][:, P * kt:P * (kt + 1)],
                                wv_sb[fi],
                                start=(fi == 0), stop=(fi == 1))
                    nc.vector.tensor_copy(
                        V[:, 4 * kq:4 * (kq + 1), :].rearrange("p a b -> p (a b)"),
                        psf)

                # attention: scores_T[k, q] blocks; e = exp(scale*s) in bf16;
                # causal masking on the diagonal via affine_select zeroing.
                for j in range(NJ):
                    _flush_cb()
                    po = ps_o.tile([P, 2, QST], F32, tag="po")
                    pd = ps_den.tile([P, QST], F32, tag="pd")
                    kt0 = 4 * j

                    pending = []  # deferred PV emitters (software pipeline)

                    def flush():
                        while pending:
                            pending.pop(0)()

                    def emit_pv(kts, ev, e01v, pdsl, first_pd, last, pd_stop):
                        # kts: list of (slot, kt); ev: e view [P, n, HQ|QST];
                        # e01v: summed-pairs views for pd; pdsl: pd column
                        # slice; last: k-tile index at which to stop.
                        def go():
                            for i, ev01 in enumerate(e01v):
                                nc.tensor.matmul(
                                    pd[:, pdsl], ones_col, ev01,
                                    start=(first_pd and i == 0),
                                    stop=(pd_stop and i == len(e01v) - 1),
                                    skip_group_check=True)
                            for t, kt in kts:
                                for c in range(2):
                                    nc.tensor.matmul(
                                        po[:, c, pdsl],
                                        V[:, kt, P * c:P * (c + 1)],
                                        ev[:, t, :],
                                        start=(kt == 0), stop=(kt == last),
                                        skip_group_check=True)
                        pending.append(go)

                    full = slice(0, QST)
                    loq = slice(0, HQ)
                    hiq = slice(HQ, QST)

                    # off-diagonal pairs, full 512-wide q
                    for g in range(2 * j):
                        psc = ps_sc.tile([P, 2, QST], F32, tag="psc")
                        for idx in range(2):
                            kt = 2 * g + idx
                            nc.tensor.matmul(
                                psc[:, idx, :],
                                HT[0][:, P * kt:P * (kt + 1)],
                                xT[0][:, QST * j:QST * (j + 1)],
                                start=True, stop=False)
                            nc.tensor.matmul(
                                psc[:, idx, :],
                                HT[1][:, P * kt:P * (kt + 1)],
                                xT[1][:, QST * j:QST * (j + 1)],
                                start=False, stop=True)
                        e = e_pool.tile([P, 2, QST], BF16, tag="e")
                        nc.scalar.activation(e, psc, AF.Exp, scale=SCALE)
                        e01 = e01_pool.tile([P, QST], BF16, tag="e01")
                        nc.vector.tensor_add(e01, e[:, 0, :], e[:, 1, :])
                        flush()
                        emit_pv([(0, 2 * g), (1, 2 * g + 1)], e, [e01],
                                full, g == 0, 4 * j + 3, False)

                    # diagonal region: two 256-wide q halves
                    # half A: q cols [0,256) of the supertile; k-tiles kt0,kt0+1
                    pscA = ps_sc.tile([P, 2, QST], F32, tag="psc")
                    vA = pscA[:, :, 0:HQ]
                    for idx in range(2):
                        kt = kt0 + idx
                        nc.tensor.matmul(
                            vA[:, idx, :],
                            HT[0][:, P * kt:P * (kt + 1)],
                            xT[0][:, QST * j:QST * j + HQ],
                            start=True, stop=False)
                        nc.tensor.matmul(
                            vA[:, idx, :],
                            HT[1][:, P * kt:P * (kt + 1)],
                            xT[1][:, QST * j:QST * j + HQ],
                            start=False, stop=True)
                    eA = e_pool.tile([P, 2, QST], BF16, tag="e")
                    eAv = eA[:, :, 0:HQ]
                    nc.scalar.activation(eAv, vA, AF.Exp, scale=SCALE)
                    # keep q >= k: col >= row (+128 for the second k-tile)
                    nc.gpsimd.affine_select(
                        out=eAv[:, 0, :], in_=eAv[:, 0, :], compare_op=GE,
                        fill=0.0, base=0, channel_multiplier=-1,
                        pattern=[[1, HQ]])
                    nc.gpsimd.affine_select(
                        out=eAv[:, 1, :], in_=eAv[:, 1, :], compare_op=GE,
                        fill=0.0, base=-P, channel_multiplier=-1,
                        pattern=[[1, HQ]])
                    e01A = e01_pool.tile([P, QST], BF16, tag="e01")
                    nc.vector.tensor_add(
                        e01A[:, 0:HQ], eAv[:, 0, :], eAv[:, 1, :])
                    flush()

                    # half B: q cols [256,512); k-tiles kt0..kt0+3
                    # (kt0, kt0+1 unmasked; kt0+2, kt0+3 triangular)
                    pscB = ps_sc.tile([P, 2, QST], F32, tag="psc")
                    vB = pscB.rearrange("p a (b q) -> p (a b) q", b=2)
                    for t in range(4):
                        kt = kt0 + t
                        nc.tensor.matmul(
                            vB[:, t, :],
                            HT[0][:, P * kt:P * (kt + 1)],
                            xT[0][:, QST * j + HQ:QST * (j + 1)],
                            start=True, stop=False)
                        nc.tensor.matmul(
                            vB[:, t, :],
                            HT[1][:, P * kt:P * (kt + 1)],
                            xT[1][:, QST * j + HQ:QST * (j + 1)],
                            start=False, stop=True)
                    eB = e_pool.tile([P, 2, QST], BF16, tag="e")
                    eBv = eB.rearrange("p a (b q) -> p (a b) q", b=2)
                    nc.scalar.activation(eBv, vB, AF.Exp, scale=SCALE)
                    nc.gpsimd.affine_select(
                        out=eBv[:, 2, :], in_=eBv[:, 2, :], compare_op=GE,
                        fill=0.0, base=0, channel_multiplier=-1,
                        pattern=[[1, HQ]])
                    nc.gpsimd.affine_select(
                        out=eBv[:, 3, :], in_=eBv[:, 3, :], compare_op=GE,
                        fill=0.0, base=-P, channel_multiplier=-1,
                        pattern=[[1, HQ]])
                    e01B = e01_pool.tile([P, QST], BF16, tag="e01")
                    nc.vector.tensor_add(
                        e01B[:, 0:HQ], eBv[:, 0, :], eBv[:, 1, :])
                    nc.vector.tensor_add(
                        e01B[:, HQ:QST], eBv[:, 2, :], eBv[:, 3, :])

                    emit_pv([(0, kt0), (1, kt0 + 1)], eAv,
                            [e01A[:, 0:HQ]], loq, j == 0, kt0 + 1, True)
                    emit_pv([(t, kt0 + t) for t in range(4)], eBv,
                            [e01B[:, 0:HQ], e01B[:, HQ:QST]],
                            hiq, j == 0, kt0 + 3, True)
                    flush()

                    for c in range(2):
                        nc.vector.tensor_copy(
                            oT[:, s, c, QST * j:QST * (j + 1)], po[:, c, :])
                    nc.vector.tensor_copy(
                        den[:, s, QST * j:QST * (j + 1)], pd[0:1, :])

                    if s in (0, 1):
                        _combine_chunk(s, j)
            _flush_cb()

        body(0)


_NC_CACHE = None


def _get_nc():
    global _NC_CACHE
    if _NC_CACHE is None:
        nc = bacc.Bacc("TRN2", target_bir_lowering=False, debug=False,
                       num_devices=8)
        with tile.TileContext(nc) as tc:
            _emit(tc)
        nc.compile()
        _NC_CACHE = nc
    return _NC_CACHE


def _make_in_maps(x, Wq, Wk, Wv, reps=1):
    bf = ml_dtypes.bfloat16
    # wall[p, i, :]: i=0,1 -> wqT chunks, 2,3 -> wkT, 4,5 -> wv (row chunks)
    wall = np.empty((P, 6, C), dtype=np.float32)
    wqT, wkT = Wq.T, Wk.T
    for i in range(2):
        wall[:, 0 + i, :] = wqT[P * i:P * (i + 1), :]
        wall[:, 2 + i, :] = wkT[P * i:P * (i + 1), :]
        wall[:, 4 + i, :] = Wv[P * i:P * (i + 1), :]
    wall = np.ascontiguousarray(wall).astype(bf)
    in_maps = []
    for core in range(8):
        b, h = core // 2, core % 2
        xb = x[core // 2]                          # [8192, 256]
        xa = xb[HALF * h:HALF * (h + 1)]           # [4096, 256]
        segs = [
            xa[0:SEG],                             # config1 seg 2h
            xa[SEG:2 * SEG],                       # config1 seg 2h+1
            xa[0::2],                              # config2 seg h
            xb[0::4],                              # config3 (full)
        ]
        xsT = np.ascontiguousarray(
            np.stack([s.T for s in segs], axis=0)).astype(bf)
        in_maps.append({
            "xsT": xsT,
            "wall": wall,
            "meta": np.array([[(SEG // 2) * h]], dtype=np.int32),
        })
    return in_maps


def run_cores(x, Wq, Wk, Wv, reps=1):
    nc = _get_nc()
    in_maps = _make_in_maps(x, Wq, Wk, Wv, reps=reps)
    res = run_bass_kernel_spmd(nc, in_maps, core_ids=list(range(8)))
    return res


def kernel(x, Wq, Wk, Wv):
    x = np.asarray(x, dtype=np.float32)
    res = run_cores(x, np.asarray(Wq, np.float32), np.asarray(Wk, np.float32),
                    np.asarray(Wv, np.float32))
    out = np.empty((B, N, D), dtype=np.float32)
    for core in range(8):
        b, h = core // 2, core % 2
        out[b, HALF * h:HALF * (h + 1), :] = \
            res.results[core]["outT"].astype(np.float32).T
    return out


# revision 4
# speedup vs baseline: 1.3520x; 1.1326x over previous
"""Dilated self-attention Trainium2 kernel (8-core SPMD), v2.

Problem (hardcoded): x [4, 8192, 256], Wq/Wk/Wv [256, 256] f32.
WS=[2048,4096,8192], RS=[1,2,4], HEAD_IDX=0 -> every config has segment
length 2048 after dilation; 28 segments total.

Sharding: core = (b, h) with b in 0..3, h in 0..1. Core (b,h) owns output
tokens [4096h, 4096h+4096) of batch b and computes the 4 attention
segments that contribute to them (seg3 = config3 computed fully on both
cores of a pair; each uses its half of the rows via a runtime offset).

Optimizations vs the f32r v1 (305us -> ~234us profiled single-shot;
PE-bound at ~80% TensorMatrix occupancy):
  - all matmul operands in bf16 (halves LDWEIGHTS time + input DMA);
    psum stays f32.  exp/softmax weights tolerate bf16 easily at the
    2e-2 rel-err gate (measured rel_fro ~4.6e-3).
  - hybrid causal blocking: off-diagonal k-tiles at 512-wide q blocks;
    the 4-k-tile diagonal region is split into two 256-wide q halves so
    ~25% of diagonal-block matmul rows are skipped instead of masked.
  - causal masking via gpsimd.affine_select zeroing e in SBUF (frees
    the PE mask matmuls entirely; no mask constants, no identity).
  - denominator: e pairs are pre-summed on DVE (bf16, cheap) so the
    ones-matmul runs once per pair instead of once per k-tile.
  - software pipelining: PV matmuls of pair g are emitted after the
    score matmuls of pair g+1 so the PE never waits on the Exp.
  - combine fused per 512-token chunk right after that chunk's PV
    finishes; reciprocal via reciprocal_approx_fast on the [1,512] den
    row, broadcast across partitions with a PE ones-matmul (gpsimd
    partition_broadcast head-of-line-blocks the affine_selects); the
    broadcast+divide+store half is deferred one j-iteration so the PE
    queue always has score matmuls ahead of it.
  - batched input DMAs (each dma_start costs ~0.6us of issue time on
    the sync engine): one packed weights tensor, xT prefetch bufs=4.
  - outputs written in bf16 (halved output DMA), upcast on host.

Timing note: device clock state varies ~20% between sessions; only
back-to-back in-session comparisons are meaningful.
"""

import numpy as np
import ml_dtypes

import concourse.bass as bass
import concourse.mybir as mybir
import concourse.tile as tile
from concourse import bacc
from concourse.bass_utils import run_bass_kernel_spmd

F32 = mybir.dt.float32
BF16 = mybir.dt.bfloat16
I32 = mybir.dt.int32
AF = mybir.ActivationFunctionType
GE = mybir.AluOpType.is_ge

B, N, C, D = 4, 8192, 256, 256
SEG = 2048          # segment length (rows) for every config
P = 128             # partitions
NT = SEG // P       # 16 k-tiles per segment
QST = 512           # q supertile width
HQ = QST // 2       # 256: diagonal-region q half width
NJ = SEG // QST     # 4 q supertiles per segment
HALF = N // 2       # 4096 tokens owned per core
NSEG = 4            # segments per core
SCALE = 1.0 / 16.0  # 1/sqrt(D)


def _emit(tc):
    nc = tc.nc

    xsT_d = nc.dram_tensor("xsT", [NSEG, C, SEG], BF16, kind="ExternalInput").ap()
    # wall: [128, 6, 256] partition-major pack of wqT(2) | wkT(2) | wv(2)
    wall_d = nc.dram_tensor("wall", [P, 6, C], BF16, kind="ExternalInput").ap()
    meta_d = nc.dram_tensor("meta", [1, 1], I32, kind="ExternalInput").ap()
    outT_d = nc.dram_tensor("outT", [C, HALF], BF16, kind="ExternalOutput").ap()

    import contextlib
    ctx = contextlib.ExitStack()
    with ctx:
        consts = ctx.enter_context(tc.tile_pool(name="consts", bufs=1))
        big = ctx.enter_context(tc.tile_pool(name="big", bufs=1))
        xt_pool = ctx.enter_context(tc.tile_pool(name="xt", bufs=2))
        e_pool = ctx.enter_context(tc.tile_pool(name="e", bufs=4))
        e01_pool = ctx.enter_context(tc.tile_pool(name="e01", bufs=4))
        pr_pool = ctx.enter_context(tc.tile_pool(name="pr", bufs=2))
        ob_pool = ctx.enter_context(tc.tile_pool(name="ob", bufs=3))
        ps_sc = ctx.enter_context(tc.tile_pool(name="ps_sc", bufs=2, space="PSUM"))
        ps_o = ctx.enter_context(tc.tile_pool(name="ps_o", bufs=1, space="PSUM"))
        ps_den = ctx.enter_context(tc.tile_pool(name="ps_den", bufs=1, space="PSUM"))
        ps_pr = ctx.enter_context(tc.tile_pool(name="ps_pr", bufs=1, space="PSUM"))

        # ---- constants (batched DMAs: issue cost on sync is ~0.6us each) ----
        meta_sb = consts.tile([1, 1], I32, tag="meta")
        nc.sync.dma_start(meta_sb, meta_d)
        wall = consts.tile([P, 6, C], BF16, tag="wall")
        nc.sync.dma_start(wall, wall_d)
        wqT_sb = [wall[:, i, :] for i in range(2)]
        wkT_sb = [wall[:, 2 + i, :] for i in range(2)]
        wv_sb = [wall[:, 4 + i, :] for i in range(2)]
        c3off_sb = meta_sb[:, 0:1]

        ones_col = consts.tile([P, P], BF16, tag="ones_col")
        nc.vector.memset(ones_col, 1.0)
        ones_row_f = consts.tile([1, P], F32, tag="ones_row_f")
        nc.vector.memset(ones_row_f, 1.0)
        ones_row = consts.tile([1, P], mybir.dt.float32r, tag="ones_row")
        nc.vector.tensor_copy(ones_row, ones_row_f)

        # GT = Wk @ Wq^T  [256, 256]  (= (Wq Wk^T)^T)
        GT_sb = [consts.tile([P, C], BF16, tag=f"GT{i}", name=f"GT{i}") for i in range(2)]
        for a in range(2):  # output row chunk
            ps = ps_sc.tile([P, 2, QST], F32, tag="psc", name="gtps")[:, 0, 0:C]
            for dch in range(2):
                nc.tensor.matmul(
                    ps, wkT_sb[dch][:, P * a:P * (a + 1)], wqT_sb[dch],
                    start=(dch == 0), stop=(dch == 1))
            nc.vector.tensor_copy(GT_sb[a], ps)

        # ---- persistent per-iteration state ----
        # oT[s][c]: unnormalized attention output, transposed: [128, 2048] per
        # (segment s, feature chunk c). den[s]: [1, 2048].
        oT = big.tile([P, NSEG, 2, SEG], F32, tag="oT")
        den = big.tile([1, NSEG, SEG], F32, tag="den")

        c3v = nc.values_load(c3off_sb, min_val=0, max_val=SEG // 2, skip_runtime_bounds_check=True)

        pending_cb = []

        def _flush_cb():
            while pending_cb:
                pending_cb.pop(0)()

        def _combine_chunk(s, j):
            # fold config2 (even tokens) and config3 (every 4th) into chunk j
            # of seg s, divide by the summed denominator, store (bf16).
            # The division/DMA half is deferred one j-iteration so its PE
            # broadcast matmul queues behind fresh score matmuls instead of
            # head-of-line-blocking the PE on the DVE chain.
            lo = QST * j
            g = SEG * s + lo            # token offset inside the half
            dstd = den[:, s, lo:lo + QST]
            dd2 = dstd.rearrange("p (q two) -> p q two", two=2)[:, :, 0:1]
            nc.vector.tensor_add(
                dd2, dd2,
                den[:, 2, g // 2:g // 2 + QST // 2].rearrange(
                    "p (q one) -> p q one", one=1))
            dd4 = dstd.rearrange("p (q four) -> p q four", four=4)[:, :, 0:1]
            nc.vector.tensor_add(
                dd4, dd4,
                den[:, 3, bass.ds(c3v + g // 4, QST // 4)].rearrange(
                    "p (q one) -> p q one", one=1))
            pr1 = pr_pool.tile([1, QST], F32, tag="pr1", name="pr1")
            nc.vector.reciprocal_approx_fast(pr1, dstd)
            pr1r = pr_pool.tile([1, QST], mybir.dt.float32r, tag="pr1r",
                                name="pr1r")
            nc.vector.tensor_copy(pr1r, pr1)
            # broadcast 1/den across partitions on the PE (a gpsimd
            # partition_broadcast here head-of-line-blocks the affine_selects)
            for c in range(2):
                dst = oT[:, s, c, lo:lo + QST]
                d2 = dst.rearrange("p (q two) -> p q two", two=2)[:, :, 0:1]
                nc.vector.tensor_add(
                    d2, d2,
                    oT[:, 2, c, g // 2:g // 2 + QST // 2].rearrange(
                        "p (q one) -> p q one", one=1))
                d4 = dst.rearrange("p (q four) -> p q four", four=4)[:, :, 0:1]
                nc.vector.tensor_add(
                    d4, d4,
                    oT[:, 3, c, bass.ds(c3v + g // 4, QST // 4)].rearrange(
                        "p (q one) -> p q one", one=1))

            def part_b():
                pr_ps = ps_pr.tile([P, QST], F32, tag="prps")
                nc.tensor.matmul(pr_ps, ones_row, pr1r, start=True, stop=True)
                for c in range(2):
                    ob = ob_pool.tile([P, QST], BF16, tag="ob", name=f"ob{c}")
                    nc.vector.tensor_mul(ob, oT[:, s, c, lo:lo + QST], pr_ps)
                    nc.sync.dma_start(outT_d[P * c:P * (c + 1), g:g + QST], ob)
            pending_cb.append(part_b)

        def _seg_ht(xT, HT, rps):
            # HT = G @ X^T ; each rp covers 1024 columns
            for fo in range(2):
                for rp in rps:
                    ps = ps_sc.tile([P, 2, QST], F32, tag="psc")
                    for idx in range(2):
                        r = 2 * rp + idx
                        for fi in range(2):
                            nc.tensor.matmul(
                                ps[:, idx, :],
                                GT_sb[fi][:, P * fo:P * (fo + 1)],
                                xT[fi][:, QST * r:QST * (r + 1)],
                                start=(fi == 0), stop=(fi == 1))
                    nc.vector.tensor_copy(
                        HT[fo][:, QST * 2 * rp:QST * 2 * (rp + 1)],
                        ps.rearrange("p a b -> p (a b)"))

        def _seg_v(xT, V, kqs):
            # V = X @ Wv ; each kq covers 4 token-major k-tiles
            for kq in kqs:
                ps = ps_sc.tile([P, 2, QST], F32, tag="psc")
                psf = ps.rearrange("p a b -> p (a b)")
                for idx in range(4):
                    kt = 4 * kq + idx
                    for fi in range(2):
                        nc.tensor.matmul(
                            psf[:, D * idx:D * (idx + 1)],
                            xT[fi][:, P * kt:P * (kt + 1)],
                            wv_sb[fi],
                            start=(fi == 0), stop=(fi == 1))
                nc.vector.tensor_copy(
                    V[:, 4 * kq:4 * (kq + 1), :].rearrange("p a b -> p (a b)"),
                    psf)

        def _seg_attn(s, xT, HT, V, js):
                # attention: scores_T[k, q] blocks; e = exp(scale*s) in bf16;
                # causal masking on the diagonal via affine_select zeroing.
                for j in js:
                    _flush_cb()
                    po = ps_o.tile([P, 2, QST], F32, tag="po")
                    pd = ps_den.tile([P, QST], F32, tag="pd")
                    kt0 = 4 * j

                    pending = []  # deferred PV emitters (software pipeline)

                    def flush():
                        while pending:
                            pending.pop(0)()

                    def emit_pv(kts, ev, e01v, pdsl, first_pd, last, pd_stop):
                        # kts: list of (slot, kt); ev: e view [P, n, HQ|QST];
                        # e01v: summed-pairs views for pd; pdsl: pd column
                        # slice; last: k-tile index at which to stop.
                        def go():
                            for i, ev01 in enumerate(e01v):
                                nc.tensor.matmul(
                                    pd[:, pdsl], ones_col, ev01,
                                    start=(first_pd and i == 0),
                                    stop=(pd_stop and i == len(e01v) - 1),
                                    skip_group_check=True)
                            for t, kt in kts:
                                for c in range(2):
                                    nc.tensor.matmul(
                                        po[:, c, pdsl],
                                        V[:, kt, P * c:P * (c + 1)],
                                        ev[:, t, :],
                                        start=(kt == 0), stop=(kt == last),
                                        skip_group_check=True)
                        pending.append(go)

                    full = slice(0, QST)
                    loq = slice(0, HQ)
                    hiq = slice(HQ, QST)

                    # off-diagonal pairs, full 512-wide q
                    for g in range(2 * j):
                        psc = ps_sc.tile([P, 2, QST], F32, tag="psc")
                        for idx in range(2):
                            kt = 2 * g + idx
                            nc.tensor.matmul(
                                psc[:, idx, :],
                                HT[0][:, P * kt:P * (kt + 1)],
                                xT[0][:, QST * j:QST * (j + 1)],
                                start=True, stop=False)
                            nc.tensor.matmul(
                                psc[:, idx, :],
                                HT[1][:, P * kt:P * (kt + 1)],
                                xT[1][:, QST * j:QST * (j + 1)],
                                start=False, stop=True)
                        e = e_pool.tile([P, 2, QST], BF16, tag="e")
                        nc.scalar.activation(e, psc, AF.Exp, scale=SCALE)
                        e01 = e01_pool.tile([P, QST], BF16, tag="e01")
                        nc.vector.tensor_add(e01, e[:, 0, :], e[:, 1, :])
                        flush()
                        emit_pv([(0, 2 * g), (1, 2 * g + 1)], e, [e01],
                                full, g == 0, 4 * j + 3, False)

                    # diagonal region: two 256-wide q halves
                    # half A: q cols [0,256) of the supertile; k-tiles kt0,kt0+1
                    pscA = ps_sc.tile([P, 2, QST], F32, tag="psc")
                    vA = pscA[:, :, 0:HQ]
                    for idx in range(2):
                        kt = kt0 + idx
                        nc.tensor.matmul(
                            vA[:, idx, :],
                            HT[0][:, P * kt:P * (kt + 1)],
                            xT[0][:, QST * j:QST * j + HQ],
                            start=True, stop=False)
                        nc.tensor.matmul(
                            vA[:, idx, :],
                            HT[1][:, P * kt:P * (kt + 1)],
                            xT[1][:, QST * j:QST * j + HQ],
                            start=False, stop=True)
                    eA = e_pool.tile([P, 2, QST], BF16, tag="e")
                    eAv = eA[:, :, 0:HQ]
                    nc.scalar.activation(eAv, vA, AF.Exp, scale=SCALE)
                    # keep q >= k: col >= row (+128 for the second k-tile)
                    nc.gpsimd.affine_select(
                        out=eAv[:, 0, :], in_=eAv[:, 0, :], compare_op=GE,
                        fill=0.0, base=0, channel_multiplier=-1,
                        pattern=[[1, HQ]])
                    nc.gpsimd.affine_select(
                        out=eAv[:, 1, :], in_=eAv[:, 1, :], compare_op=GE,
                        fill=0.0, base=-P, channel_multiplier=-1,
                        pattern=[[1, HQ]])
                    e01A = e01_pool.tile([P, QST], BF16, tag="e01")
                    nc.vector.tensor_add(
                        e01A[:, 0:HQ], eAv[:, 0, :], eAv[:, 1, :])
                    flush()

                    # half B: q cols [256,512); k-tiles kt0..kt0+3
                    # (kt0, kt0+1 unmasked; kt0+2, kt0+3 triangular)
                    pscB = ps_sc.tile([P, 2, QST], F32, tag="psc")
                    vB = pscB.rearrange("p a (b q) -> p (a b) q", b=2)
                    for t in range(4):
                        kt = kt0 + t
                        nc.tensor.matmul(
                            vB[:, t, :],
                            HT[0][:, P * kt:P * (kt + 1)],
                            xT[0][:, QST * j + HQ:QST * (j + 1)],
                            start=True, stop=False)
                        nc.tensor.matmul(
                            vB[:, t, :],
                            HT[1][:, P * kt:P * (kt + 1)],
                            xT[1][:, QST * j + HQ:QST * (j + 1)],
                            start=False, stop=True)
                    eB = e_pool.tile([P, 2, QST], BF16, tag="e")
                    eBv = eB.rearrange("p a (b q) -> p (a b) q", b=2)
                    nc.scalar.activation(eBv, vB, AF.Exp, scale=SCALE)
                    nc.gpsimd.affine_select(
                        out=eBv[:, 2, :], in_=eBv[:, 2, :], compare_op=GE,
                        fill=0.0, base=0, channel_multiplier=-1,
                        pattern=[[1, HQ]])
                    nc.gpsimd.affine_select(
                        out=eBv[:, 3, :], in_=eBv[:, 3, :], compare_op=GE,
                        fill=0.0, base=-P, channel_multiplier=-1,
                        pattern=[[1, HQ]])
                    e01B = e01_pool.tile([P, QST], BF16, tag="e01")
                    nc.vector.tensor_add(
                        e01B[:, 0:HQ], eBv[:, 0, :], eBv[:, 1, :])
                    nc.vector.tensor_add(
                        e01B[:, HQ:QST], eBv[:, 2, :], eBv[:, 3, :])

                    emit_pv([(0, kt0), (1, kt0 + 1)], eAv,
                            [e01A[:, 0:HQ]], loq, j == 0, kt0 + 1, True)
                    emit_pv([(t, kt0 + t) for t in range(4)], eBv,
                            [e01B[:, 0:HQ], e01B[:, HQ:QST]],
                            hiq, j == 0, kt0 + 3, True)
                    flush()

                    for c in range(2):
                        nc.vector.tensor_copy(
                            oT[:, s, c, QST * j:QST * (j + 1)], po[:, c, :])
                    nc.vector.tensor_copy(
                        den[:, s, QST * j:QST * (j + 1)], pd[0:1, :])

                    if s in (0, 1):
                        _combine_chunk(s, j)

        def body(_iv):
            for s in (2, 3, 0, 1):
                # load x^T for this segment
                xT = [xt_pool.tile([P, SEG], BF16, tag=f"xT{c}", name=f"xT{c}", bufs=4) for c in range(2)]
                for c in range(2):
                    nc.sync.dma_start(xT[c], xsT_d[s, P * c:P * (c + 1), :])
                HT = [xt_pool.tile([P, SEG], BF16, tag=f"HT{c}", name=f"HT{c}", bufs=1) for c in range(2)]
                V = xt_pool.tile([P, NT, D], BF16, tag="V", bufs=1)

                if s == 3:
                    # config-3 segment: split by output ownership instead of
                    # computing all 2048 queries on both cores of the pair.
                    # h0 (c3off==0) owns q supertiles 0,1 and needs only
                    # k-tiles 0..7; h1 owns supertiles 2,3 and needs all 16.
                    with tc.If(c3v == 0):
                        _seg_ht(xT, HT, [0])
                        _seg_v(xT, V, [0, 1])
                        _seg_attn(s, xT, HT, V, [0, 1])
                    with tc.If(c3v == SEG // 2):
                        _seg_ht(xT, HT, [0, 1])
                        _seg_v(xT, V, [0, 1, 2, 3])
                        _seg_attn(s, xT, HT, V, [2, 3])
                else:
                    _seg_ht(xT, HT, [0, 1])
                    _seg_v(xT, V, [0, 1, 2, 3])
                    _seg_attn(s, xT, HT, V, list(range(NJ)))
            _flush_cb()

        body(0)


_NC_CACHE = None


def _get_nc():
    global _NC_CACHE
    if _NC_CACHE is None:
        nc = bacc.Bacc("TRN2", target_bir_lowering=False, debug=False,
                       num_devices=8)
        with tile.TileContext(nc) as tc:
            _emit(tc)
        nc.compile()
        _NC_CACHE = nc
    return _NC_CACHE


def _make_in_maps(x, Wq, Wk, Wv, reps=1):
    bf = ml_dtypes.bfloat16
    # wall[p, i, :]: i=0,1 -> wqT chunks, 2,3 -> wkT, 4,5 -> wv (row chunks)
    wall = np.empty((P, 6, C), dtype=np.float32)
    wqT, wkT = Wq.T, Wk.T
    for i in range(2):
        wall[:, 0 + i, :] = wqT[P * i:P * (i + 1), :]
        wall[:, 2 + i, :] = wkT[P * i:P * (i + 1), :]
        wall[:, 4 + i, :] = Wv[P * i:P * (i + 1), :]
    wall = np.ascontiguousarray(wall).astype(bf)
    in_maps = []
    for core in range(8):
        b, h = core // 2, core % 2
        xb = x[core // 2]                          # [8192, 256]
        xa = xb[HALF * h:HALF * (h + 1)]           # [4096, 256]
        segs = [
            xa[0:SEG],                             # config1 seg 2h
            xa[SEG:2 * SEG],                       # config1 seg 2h+1
            xa[0::2],                              # config2 seg h
            xb[0::4],                              # config3 (full)
        ]
        xsT = np.ascontiguousarray(
            np.stack([s.T for s in segs], axis=0)).astype(bf)
        in_maps.append({
            "xsT": xsT,
            "wall": wall,
            "meta": np.array([[(SEG // 2) * h]], dtype=np.int32),
        })
    return in_maps


def run_cores(x, Wq, Wk, Wv, reps=1):
    nc = _get_nc()
    in_maps = _make_in_maps(x, Wq, Wk, Wv, reps=reps)
    res = run_bass_kernel_spmd(nc, in_maps, core_ids=list(range(8)))
    return res


def kernel(x, Wq, Wk, Wv):
    x = np.asarray(x, dtype=np.float32)
    res = run_cores(x, np.asarray(Wq, np.float32), np.asarray(Wk, np.float32),
                    np.asarray(Wv, np.float32))
    out = np.empty((B, N, D), dtype=np.float32)
    for core in range(8):
        b, h = core // 2, core % 2
        out[b, HALF * h:HALF * (h + 1), :] = \
            res.results[core]["outT"].astype(np.float32).T
    return out


# revision 5
# speedup vs baseline: 1.3540x; 1.0015x over previous
"""Dilated self-attention Trainium2 kernel (8-core SPMD), v2.

Problem (hardcoded): x [4, 8192, 256], Wq/Wk/Wv [256, 256] f32.
WS=[2048,4096,8192], RS=[1,2,4], HEAD_IDX=0 -> every config has segment
length 2048 after dilation; 28 segments total.

Sharding: core = (b, h) with b in 0..3, h in 0..1. Core (b,h) owns output
tokens [4096h, 4096h+4096) of batch b and computes the 4 attention
segments that contribute to them (seg3 = config3 computed fully on both
cores of a pair; each uses its half of the rows via a runtime offset).

Optimizations vs the f32r v1 (305us -> ~234us profiled single-shot;
PE-bound at ~80% TensorMatrix occupancy):
  - all matmul operands in bf16 (halves LDWEIGHTS time + input DMA);
    psum stays f32.  exp/softmax weights tolerate bf16 easily at the
    2e-2 rel-err gate (measured rel_fro ~4.6e-3).
  - hybrid causal blocking: off-diagonal k-tiles at 512-wide q blocks;
    the 4-k-tile diagonal region is split into two 256-wide q halves so
    ~25% of diagonal-block matmul rows are skipped instead of masked.
  - causal masking via gpsimd.affine_select zeroing e in SBUF (frees
    the PE mask matmuls entirely; no mask constants, no identity).
  - denominator: e pairs are pre-summed on DVE (bf16, cheap) so the
    ones-matmul runs once per pair instead of once per k-tile.
  - software pipelining: PV matmuls of pair g are emitted after the
    score matmuls of pair g+1 so the PE never waits on the Exp.
  - combine fused per 512-token chunk right after that chunk's PV
    finishes; reciprocal via reciprocal_approx_fast on the [1,512] den
    row, broadcast across partitions with a PE ones-matmul (gpsimd
    partition_broadcast head-of-line-blocks the affine_selects); the
    broadcast+divide+store half is deferred one j-iteration so the PE
    queue always has score matmuls ahead of it.
  - batched input DMAs (each dma_start costs ~0.6us of issue time on
    the sync engine): one packed weights tensor, xT prefetch bufs=4.
  - outputs written in bf16 (halved output DMA), upcast on host.

Timing note: device clock state varies ~20% between sessions; only
back-to-back in-session comparisons are meaningful.
"""

import numpy as np
import ml_dtypes

import concourse.bass as bass
import concourse.mybir as mybir
import concourse.tile as tile
from concourse import bacc
from concourse.bass_utils import run_bass_kernel_spmd

F32 = mybir.dt.float32
BF16 = mybir.dt.bfloat16
I32 = mybir.dt.int32
AF = mybir.ActivationFunctionType
GE = mybir.AluOpType.is_ge

B, N, C, D = 4, 8192, 256, 256
SEG = 2048          # segment length (rows) for every config
P = 128             # partitions
NT = SEG // P       # 16 k-tiles per segment
QST = 512           # q supertile width
HQ = QST // 2       # 256: diagonal-region q half width
NJ = SEG // QST     # 4 q supertiles per segment
HALF = N // 2       # 4096 tokens owned per core
NSEG = 4            # segments per core
SCALE = 1.0 / 16.0  # 1/sqrt(D)


def _emit(tc):
    nc = tc.nc

    xsT_d = nc.dram_tensor("xsT", [NSEG, C, SEG], BF16, kind="ExternalInput").ap()
    # wall: [128, 6, 256] partition-major pack of wqT(2) | wkT(2) | wv(2)
    wall_d = nc.dram_tensor("wall", [P, 6, C], BF16, kind="ExternalInput").ap()
    meta_d = nc.dram_tensor("meta", [1, 1], I32, kind="ExternalInput").ap()
    outT_d = nc.dram_tensor("outT", [C, HALF], BF16, kind="ExternalOutput").ap()

    import contextlib
    ctx = contextlib.ExitStack()
    with ctx:
        consts = ctx.enter_context(tc.tile_pool(name="consts", bufs=1))
        big = ctx.enter_context(tc.tile_pool(name="big", bufs=1))
        xt_pool = ctx.enter_context(tc.tile_pool(name="xt", bufs=2))
        e_pool = ctx.enter_context(tc.tile_pool(name="e", bufs=4))
        e01_pool = ctx.enter_context(tc.tile_pool(name="e01", bufs=4))
        pr_pool = ctx.enter_context(tc.tile_pool(name="pr", bufs=2))
        ob_pool = ctx.enter_context(tc.tile_pool(name="ob", bufs=3))
        ps_sc = ctx.enter_context(tc.tile_pool(name="ps_sc", bufs=2, space="PSUM"))
        ps_o = ctx.enter_context(tc.tile_pool(name="ps_o", bufs=1, space="PSUM"))
        ps_den = ctx.enter_context(tc.tile_pool(name="ps_den", bufs=1, space="PSUM"))
        ps_pr = ctx.enter_context(tc.tile_pool(name="ps_pr", bufs=1, space="PSUM"))

        # ---- constants (batched DMAs: issue cost on sync is ~0.6us each) ----
        meta_sb = consts.tile([1, 1], I32, tag="meta")
        nc.sync.dma_start(meta_sb, meta_d)
        wall = consts.tile([P, 6, C], BF16, tag="wall")
        nc.sync.dma_start(wall, wall_d)
        wqT_sb = [wall[:, i, :] for i in range(2)]
        wkT_sb = [wall[:, 2 + i, :] for i in range(2)]
        wv_sb = [wall[:, 4 + i, :] for i in range(2)]
        c3off_sb = meta_sb[:, 0:1]

        ones_col = consts.tile([P, P], BF16, tag="ones_col")
        nc.vector.memset(ones_col, 1.0)
        ones_row_f = consts.tile([1, P], F32, tag="ones_row_f")
        nc.vector.memset(ones_row_f, 1.0)
        ones_row = consts.tile([1, P], mybir.dt.float32r, tag="ones_row")
        nc.vector.tensor_copy(ones_row, ones_row_f)

        # GT = Wk @ Wq^T  [256, 256]  (= (Wq Wk^T)^T)
        GT_sb = [consts.tile([P, C], BF16, tag=f"GT{i}", name=f"GT{i}") for i in range(2)]
        for a in range(2):  # output row chunk
            ps = ps_sc.tile([P, 2, QST], F32, tag="psc", name="gtps")[:, 0, 0:C]
            for dch in range(2):
                nc.tensor.matmul(
                    ps, wkT_sb[dch][:, P * a:P * (a + 1)], wqT_sb[dch],
                    start=(dch == 0), stop=(dch == 1))
            nc.vector.tensor_copy(GT_sb[a], ps)

        # ---- persistent per-iteration state ----
        # oT[s][c]: unnormalized attention output, transposed: [128, 2048] per
        # (segment s, feature chunk c). den[s]: [1, 2048].
        oT = big.tile([P, NSEG, 2, SEG], F32, tag="oT")
        den = big.tile([1, NSEG, SEG], F32, tag="den")

        c3v = nc.values_load(c3off_sb, min_val=0, max_val=SEG // 2, skip_runtime_bounds_check=True)

        pending_cb = []

        def _flush_cb():
            while pending_cb:
                pending_cb.pop(0)()

        def _combine_chunk(s, j):
            # fold config2 (even tokens) and config3 (every 4th) into chunk j
            # of seg s, divide by the summed denominator, store (bf16).
            # The division/DMA half is deferred one j-iteration so its PE
            # broadcast matmul queues behind fresh score matmuls instead of
            # head-of-line-blocking the PE on the DVE chain.
            lo = QST * j
            g = SEG * s + lo            # token offset inside the half
            dstd = den[:, s, lo:lo + QST]
            dd2 = dstd.rearrange("p (q two) -> p q two", two=2)[:, :, 0:1]
            nc.vector.tensor_add(
                dd2, dd2,
                den[:, 2, g // 2:g // 2 + QST // 2].rearrange(
                    "p (q one) -> p q one", one=1))
            dd4 = dstd.rearrange("p (q four) -> p q four", four=4)[:, :, 0:1]
            nc.vector.tensor_add(
                dd4, dd4,
                den[:, 3, bass.ds(c3v + g // 4, QST // 4)].rearrange(
                    "p (q one) -> p q one", one=1))
            pr1 = pr_pool.tile([1, QST], F32, tag="pr1", name="pr1")
            nc.vector.reciprocal_approx_fast(pr1, dstd)
            pr1r = pr_pool.tile([1, QST], mybir.dt.float32r, tag="pr1r",
                                name="pr1r")
            nc.vector.tensor_copy(pr1r, pr1)
            # broadcast 1/den across partitions on the PE (a gpsimd
            # partition_broadcast here head-of-line-blocks the affine_selects)
            for c in range(2):
                dst = oT[:, s, c, lo:lo + QST]
                d2 = dst.rearrange("p (q two) -> p q two", two=2)[:, :, 0:1]
                nc.vector.tensor_add(
                    d2, d2,
                    oT[:, 2, c, g // 2:g // 2 + QST // 2].rearrange(
                        "p (q one) -> p q one", one=1))
                d4 = dst.rearrange("p (q four) -> p q four", four=4)[:, :, 0:1]
                nc.vector.tensor_add(
                    d4, d4,
                    oT[:, 3, c, bass.ds(c3v + g // 4, QST // 4)].rearrange(
                        "p (q one) -> p q one", one=1))

            def part_b():
                pr_ps = ps_pr.tile([P, QST], F32, tag="prps")
                nc.tensor.matmul(pr_ps, ones_row, pr1r, start=True, stop=True)
                for c in range(2):
                    ob = ob_pool.tile([P, QST], BF16, tag="ob", name=f"ob{c}")
                    nc.vector.tensor_mul(ob, oT[:, s, c, lo:lo + QST], pr_ps)
                    nc.sync.dma_start(outT_d[P * c:P * (c + 1), g:g + QST], ob)
            pending_cb.append(part_b)

        def _seg_ht(xT, HT, rps):
            # HT = G @ X^T ; each rp covers 1024 columns
            for fo in range(2):
                for rp in rps:
                    ps = ps_sc.tile([P, 2, QST], F32, tag="psc")
                    for idx in range(2):
                        r = 2 * rp + idx
                        for fi in range(2):
                            nc.tensor.matmul(
                                ps[:, idx, :],
                                GT_sb[fi][:, P * fo:P * (fo + 1)],
                                xT[fi][:, QST * r:QST * (r + 1)],
                                start=(fi == 0), stop=(fi == 1))
                    nc.vector.tensor_copy(
                        HT[fo][:, QST * 2 * rp:QST * 2 * (rp + 1)],
                        ps.rearrange("p a b -> p (a b)"))

        def _seg_v(xT, V, kqs):
            # V = X @ Wv ; each kq covers 4 token-major k-tiles
            for kq in kqs:
                ps = ps_sc.tile([P, 2, QST], F32, tag="psc")
                psf = ps.rearrange("p a b -> p (a b)")
                for idx in range(4):
                    kt = 4 * kq + idx
                    for fi in range(2):
                        nc.tensor.matmul(
                            psf[:, D * idx:D * (idx + 1)],
                            xT[fi][:, P * kt:P * (kt + 1)],
                            wv_sb[fi],
                            start=(fi == 0), stop=(fi == 1))
                nc.vector.tensor_copy(
                    V[:, 4 * kq:4 * (kq + 1), :].rearrange("p a b -> p (a b)"),
                    psf)

        def _seg_attn(s, xT, HT, V, js):
                # attention: scores_T[k, q] blocks; e = exp(scale*s) in bf16;
                # causal masking on the diagonal via affine_select zeroing.
                for j in js:
                    po = ps_o.tile([P, 2, QST], F32, tag="po")
                    pd = ps_den.tile([P, QST], F32, tag="pd")
                    kt0 = 4 * j

                    pending = []  # deferred PV emitters (software pipeline)

                    def flush():
                        while pending:
                            pending.pop(0)()

                    def emit_pv(kts, ev, e01v, pdsl, first_pd, last, pd_stop):
                        # kts: list of (slot, kt); ev: e view [P, n, HQ|QST];
                        # e01v: summed-pairs views for pd; pdsl: pd column
                        # slice; last: k-tile index at which to stop.
                        def go():
                            for i, ev01 in enumerate(e01v):
                                nc.tensor.matmul(
                                    pd[:, pdsl], ones_col, ev01,
                                    start=(first_pd and i == 0),
                                    stop=(pd_stop and i == len(e01v) - 1),
                                    skip_group_check=True)
                            for t, kt in kts:
                                for c in range(2):
                                    nc.tensor.matmul(
                                        po[:, c, pdsl],
                                        V[:, kt, P * c:P * (c + 1)],
                                        ev[:, t, :],
                                        start=(kt == 0), stop=(kt == last),
                                        skip_group_check=True)
                        pending.append(go)

                    full = slice(0, QST)
                    loq = slice(0, HQ)
                    hiq = slice(HQ, QST)

                    # off-diagonal pairs, full 512-wide q
                    for g in range(2 * j):
                        psc = ps_sc.tile([P, 2, QST], F32, tag="psc")
                        for idx in range(2):
                            kt = 2 * g + idx
                            nc.tensor.matmul(
                                psc[:, idx, :],
                                HT[0][:, P * kt:P * (kt + 1)],
                                xT[0][:, QST * j:QST * (j + 1)],
                                start=True, stop=False)
                            nc.tensor.matmul(
                                psc[:, idx, :],
                                HT[1][:, P * kt:P * (kt + 1)],
                                xT[1][:, QST * j:QST * (j + 1)],
                                start=False, stop=True)
                        e = e_pool.tile([P, 2, QST], BF16, tag="e")
                        nc.scalar.activation(e, psc, AF.Exp, scale=SCALE)
                        e01 = e01_pool.tile([P, QST], BF16, tag="e01")
                        nc.vector.tensor_add(e01, e[:, 0, :], e[:, 1, :])
                        flush()
                        emit_pv([(0, 2 * g), (1, 2 * g + 1)], e, [e01],
                                full, g == 0, 4 * j + 3, False)
                        if g == 0:
                            _flush_cb()

                    # diagonal region: two 256-wide q halves
                    # half A: q cols [0,256) of the supertile; k-tiles kt0,kt0+1
                    pscA = ps_sc.tile([P, 2, QST], F32, tag="psc")
                    vA = pscA[:, :, 0:HQ]
                    for idx in range(2):
                        kt = kt0 + idx
                        nc.tensor.matmul(
                            vA[:, idx, :],
                            HT[0][:, P * kt:P * (kt + 1)],
                            xT[0][:, QST * j:QST * j + HQ],
                            start=True, stop=False)
                        nc.tensor.matmul(
                            vA[:, idx, :],
                            HT[1][:, P * kt:P * (kt + 1)],
                            xT[1][:, QST * j:QST * j + HQ],
                            start=False, stop=True)
                    eA = e_pool.tile([P, 2, QST], BF16, tag="e")
                    eAv = eA[:, :, 0:HQ]
                    nc.scalar.activation(eAv, vA, AF.Exp, scale=SCALE)
                    # keep q >= k: col >= row (+128 for the second k-tile)
                    nc.gpsimd.affine_select(
                        out=eAv[:, 0, :], in_=eAv[:, 0, :], compare_op=GE,
                        fill=0.0, base=0, channel_multiplier=-1,
                        pattern=[[1, HQ]])
                    nc.gpsimd.affine_select(
                        out=eAv[:, 1, :], in_=eAv[:, 1, :], compare_op=GE,
                        fill=0.0, base=-P, channel_multiplier=-1,
                        pattern=[[1, HQ]])
                    e01A = e01_pool.tile([P, QST], BF16, tag="e01")
                    nc.vector.tensor_add(
                        e01A[:, 0:HQ], eAv[:, 0, :], eAv[:, 1, :])
                    flush()
                    if j == 0 or not any(True for _ in range(2 * j)):
                        _flush_cb()

                    # half B: q cols [256,512); k-tiles kt0..kt0+3
                    # (kt0, kt0+1 unmasked; kt0+2, kt0+3 triangular)
                    pscB = ps_sc.tile([P, 2, QST], F32, tag="psc")
                    vB = pscB.rearrange("p a (b q) -> p (a b) q", b=2)
                    for t in range(4):
                        kt = kt0 + t
                        nc.tensor.matmul(
                            vB[:, t, :],
                            HT[0][:, P * kt:P * (kt + 1)],
                            xT[0][:, QST * j + HQ:QST * (j + 1)],
                            start=True, stop=False)
                        nc.tensor.matmul(
                            vB[:, t, :],
                            HT[1][:, P * kt:P * (kt + 1)],
                            xT[1][:, QST * j + HQ:QST * (j + 1)],
                            start=False, stop=True)
                    eB = e_pool.tile([P, 2, QST], BF16, tag="e")
                    eBv = eB.rearrange("p a (b q) -> p (a b) q", b=2)
                    nc.scalar.activation(eBv, vB, AF.Exp, scale=SCALE)
                    nc.gpsimd.affine_select(
                        out=eBv[:, 2, :], in_=eBv[:, 2, :], compare_op=GE,
                        fill=0.0, base=0, channel_multiplier=-1,
                        pattern=[[1, HQ]])
                    nc.gpsimd.affine_select(
                        out=eBv[:, 3, :], in_=eBv[:, 3, :], compare_op=GE,
                        fill=0.0, base=-P, channel_multiplier=-1,
                        pattern=[[1, HQ]])
                    e01B = e01_pool.tile([P, QST], BF16, tag="e01")
                    nc.vector.tensor_add(
                        e01B[:, 0:HQ], eBv[:, 0, :], eBv[:, 1, :])
                    nc.vector.tensor_add(
                        e01B[:, HQ:QST], eBv[:, 2, :], eBv[:, 3, :])

                    emit_pv([(0, kt0), (1, kt0 + 1)], eAv,
                            [e01A[:, 0:HQ]], loq, j == 0, kt0 + 1, True)
                    emit_pv([(t, kt0 + t) for t in range(4)], eBv,
                            [e01B[:, 0:HQ], e01B[:, HQ:QST]],
                            hiq, j == 0, kt0 + 3, True)
                    flush()

                    for c in range(2):
                        nc.vector.tensor_copy(
                            oT[:, s, c, QST * j:QST * (j + 1)], po[:, c, :])
                    nc.vector.tensor_copy(
                        den[:, s, QST * j:QST * (j + 1)], pd[0:1, :])

                    if s in (0, 1):
                        _combine_chunk(s, j)

        def body(_iv):
            for s in (2, 3, 0, 1):
                # load x^T for this segment
                xT = [xt_pool.tile([P, SEG], BF16, tag=f"xT{c}", name=f"xT{c}", bufs=4) for c in range(2)]
                for c in range(2):
                    nc.sync.dma_start(xT[c], xsT_d[s, P * c:P * (c + 1), :])
                HT = [xt_pool.tile([P, SEG], BF16, tag=f"HT{c}", name=f"HT{c}", bufs=1) for c in range(2)]
                V = xt_pool.tile([P, NT, D], BF16, tag="V", bufs=1)

                if s == 3:
                    # config-3 segment: split by output ownership instead of
                    # computing all 2048 queries on both cores of the pair.
                    # h0 (c3off==0) owns q supertiles 0,1 and needs only
                    # k-tiles 0..7; h1 owns supertiles 2,3 and needs all 16.
                    with tc.If(c3v == 0):
                        _seg_ht(xT, HT, [0])
                        _seg_v(xT, V, [0, 1])
                        _seg_attn(s, xT, HT, V, [0, 1])
                    with tc.If(c3v == SEG // 2):
                        _seg_ht(xT, HT, [0, 1])
                        _seg_v(xT, V, [0, 1, 2, 3])
                        _seg_attn(s, xT, HT, V, [2, 3])
                else:
                    _seg_ht(xT, HT, [0, 1])
                    _seg_v(xT, V, [0, 1, 2, 3])
                    _seg_attn(s, xT, HT, V, list(range(NJ)))
            _flush_cb()

        body(0)


_NC_CACHE = None


def _get_nc():
    global _NC_CACHE
    if _NC_CACHE is None:
        nc = bacc.Bacc("TRN2", target_bir_lowering=False, debug=False,
                       num_devices=8)
        with tile.TileContext(nc) as tc:
            _emit(tc)
        nc.compile()
        _NC_CACHE = nc
    return _NC_CACHE


def _make_in_maps(x, Wq, Wk, Wv, reps=1):
    bf = ml_dtypes.bfloat16
    # wall[p, i, :]: i=0,1 -> wqT chunks, 2,3 -> wkT, 4,5 -> wv (row chunks)
    wall = np.empty((P, 6, C), dtype=np.float32)
    wqT, wkT = Wq.T, Wk.T
    for i in range(2):
        wall[:, 0 + i, :] = wqT[P * i:P * (i + 1), :]
        wall[:, 2 + i, :] = wkT[P * i:P * (i + 1), :]
        wall[:, 4 + i, :] = Wv[P * i:P * (i + 1), :]
    wall = np.ascontiguousarray(wall).astype(bf)
    in_maps = []
    for core in range(8):
        b, h = core // 2, core % 2
        xb = x[core // 2]                          # [8192, 256]
        xa = xb[HALF * h:HALF * (h + 1)]           # [4096, 256]
        segs = [
            xa[0:SEG],                             # config1 seg 2h
            xa[SEG:2 * SEG],                       # config1 seg 2h+1
            xa[0::2],                              # config2 seg h
            xb[0::4],                              # config3 (full)
        ]
        xsT = np.ascontiguousarray(
            np.stack([s.T for s in segs], axis=0)).astype(bf)
        in_maps.append({
            "xsT": xsT,
            "wall": wall,
            "meta": np.array([[(SEG // 2) * h]], dtype=np.int32),
        })
    return in_maps


def run_cores(x, Wq, Wk, Wv, reps=1):
    nc = _get_nc()
    in_maps = _make_in_maps(x, Wq, Wk, Wv, reps=reps)
    res = run_bass_kernel_spmd(nc, in_maps, core_ids=list(range(8)))
    return res


def kernel(x, Wq, Wk, Wv):
    x = np.asarray(x, dtype=np.float32)
    res = run_cores(x, np.asarray(Wq, np.float32), np.asarray(Wk, np.float32),
                    np.asarray(Wv, np.float32))
    out = np.empty((B, N, D), dtype=np.float32)
    for core in range(8):
        b, h = core // 2, core % 2
        out[b, HALF * h:HALF * (h + 1), :] = \
            res.results[core]["outT"].astype(np.float32).T
    return out
